# revision 1
# baseline (speedup 1.0000x reference)
"""Gabor layer Trainium2 kernel.

Per gabor g and pixel (x,y) the contribution is
  amp[g,c] * exp(E_g) * cos(S_g + phase[g,c])
with E quadratic and S affine in pixel coords. Using
cos(S+p) = cos(p)cos(S) - sin(p)sin(S) the channel sum over g becomes a
matmul over gauss*cos(S) / gauss*sin(S) planes (contraction = gabors).

Host-side (float64): clip/fold all parameters, cull gabors per 64-row core
strip (a gabor whose gaussian never exceeds 1e-7 in the strip is dropped;
counts are ~70 of 256 for normal inputs, padded to a 128 chunk), build
per-tile weight tables.

Device pipeline per 512-pixel tile (16x32 px), Gc = 128 gabor rows:
  PE : E = WE^T @ feat6          (K=6 fp32, tile-centered delta features --
                                  full fp32 matmul for the cancellation)
       S = WShi^T @ onehot + WSlo^T @ onehot   (K=48 bf16 x2 passes;
             S[g,p] = A[g,row(p)] + B[g,col(p)], tables wrapped to [-pi,pi)
             and hi/lo split on host, one-hot products are exact in bf16)
  ACT: gauss = Exp(E);  t2 = Sin(S*~0.5);  u = Square(t2*sqrt2) = 2*t2^2
  DVE: w1 = add_range_wrap(S) in [-pi,pi];  (ACT: sS = Sin(w1))
       p1n = (u-1)*gauss = -cos(S)*gauss;   p2 = gauss*sS
  PE : out[3,512] += [-alpha; beta]^T @ [p1n; p2]   (2x K=128 fp32)
Tiles run in blocks of B with all Exp ops grouped before the Sin ops so the
ACT table reload (1.3us) happens twice per block, not per tile (Square/Copy
are in every ACT table set).

Sharding: 8 cores x 64-row strips, no collectives; final clamp and strip
concatenation on host.
"""

import os
import sys

import numpy as np

for _p in ("/opt/trn_rl_repo",):
    if os.path.isdir(_p) and _p not in sys.path:
        sys.path.append(_p)

H = W = 512
G = 256
NCORES = 8
SH = H // NCORES      # strip rows per core
TR, TC = 16, 32       # tile rows x cols
N = TR * TC           # 512 pixels per tile
TPR = W // TC         # tiles per strip row = 16
NT = (SH // TR) * TPR # tiles per core = 64
B = 8                 # tiles per block (ACT table phase granularity)
NBLK = NT // B
KS = TR + TC          # one-hot feature rows for the S matmul
PI = float(np.pi)
SCALE_HALF = float(np.float32(0.5 * (1.0 - 2.4e-7)))
CULL_THR = 1e-7       # drop gabors whose max contribution in-strip is below

_PROGRAMS = {}


def _build_program(nchunk):
    from concourse import bacc, mybir, tile

    f32 = mybir.dt.float32
    bf16 = mybir.dt.bfloat16
    Act = mybir.ActivationFunctionType
    Alu = mybir.AluOpType
    Gc = 128 * nchunk

    nc = bacc.Bacc("TRN2", target_bir_lowering=False, debug=False,
                   num_devices=NCORES)

    featd = nc.dram_tensor("feat", [6, NT, N], f32, kind="ExternalInput")
    wed = nc.dram_tensor("we", [6, NT, Gc], f32, kind="ExternalInput")
    wshd = nc.dram_tensor("wsh", [KS, NT, Gc], bf16, kind="ExternalInput")
    wsld = nc.dram_tensor("wsl", [KS, NT, Gc], bf16, kind="ExternalInput")
    ohd = nc.dram_tensor("onehot", [KS, N], bf16, kind="ExternalInput")
    abd = nc.dram_tensor("ab", [128, nchunk * 2 * 3], f32, kind="ExternalInput")
    outd = nc.dram_tensor("out", [3, SH, W], f32, kind="ExternalOutput")

    with tile.TileContext(nc) as tc:
        with (
            tc.tile_pool(name="io", bufs=1) as iop,
            tc.tile_pool(name="gauss", bufs=B + 2) as gp,
            tc.tile_pool(name="trig", bufs=3) as trigp,
            tc.tile_pool(name="prod", bufs=3) as pp,
            tc.tile_pool(name="mm1", bufs=4, space="PSUM") as mm1p,
            tc.tile_pool(name="acc", bufs=2, space="PSUM") as accp,
        ):
            ab_sb = iop.tile([128, nchunk * 2 * 3], f32, tag="ab")
            nc.sync.dma_start(out=ab_sb[:], in_=abd[:])
            oh_sb = iop.tile([KS, N], bf16, tag="oh")
            nc.sync.dma_start(out=oh_sb[:], in_=ohd[:])

            for blk in range(NBLK):
                t0 = blk * B
                fb = iop.tile([6, B, N], f32, tag="feat", bufs=2)
                nc.sync.dma_start(out=fb[:], in_=featd[:, t0:t0 + B, :])
                we = iop.tile([6, B, Gc], f32, tag="we", bufs=2)
                nc.sync.dma_start(out=we[:], in_=wed[:, t0:t0 + B, :])
                wsh = iop.tile([KS, B, Gc], bf16, tag="wsh", bufs=2)
                nc.sync.dma_start(out=wsh[:], in_=wshd[:, t0:t0 + B, :])
                wsl = iop.tile([KS, B, Gc], bf16, tag="wsl", bufs=2)
                nc.sync.dma_start(out=wsl[:], in_=wsld[:, t0:t0 + B, :])

                # Phase A: gaussians for the whole block (Exp table loaded).
                gts = []
                for t in range(B):
                    mE = mm1p.tile([128, nchunk, N], f32, tag="m1", name="mE")
                    for c in range(nchunk):
                        nc.tensor.matmul(
                            mE[:, c, :],
                            we[:, t, c * 128:(c + 1) * 128],
                            fb[:, t, :],
                            start=True, stop=True,
                        )
                    g = gp.tile([128, nchunk, N], f32, tag="gauss", name="gauss")
                    nc.scalar.activation(g[:], mE[:], Act.Exp)
                    gts.append(g)

                # Phase B: sinusoid + products + reduction (Sin table loaded).
                for t in range(B):
                    mS = mm1p.tile([128, nchunk, N], f32, tag="m1", name="mS")
                    for c in range(nchunk):
                        nc.tensor.matmul(
                            mS[:, c, :],
                            wsh[:, t, c * 128:(c + 1) * 128],
                            oh_sb[:],
                            start=True, stop=False,
                        )
                        nc.tensor.matmul(
                            mS[:, c, :],
                            wsl[:, t, c * 128:(c + 1) * 128],
                            oh_sb[:],
                            start=False, stop=True,
                        )
                    t2 = trigp.tile([128, nchunk, N], f32, tag="t2", name="t2")
                    nc.scalar.activation(t2[:], mS[:], Act.Sin, scale=SCALE_HALF)
                    w1 = trigp.tile([128, nchunk, N], f32, tag="w1", name="w1")
                    nc.vector.add_range_wrap(w1[:], mS[:], 0.0, PI, 2.0 * PI)
                    ss = trigp.tile([128, nchunk, N], f32, tag="ss", name="ss")
                    nc.scalar.activation(ss[:], w1[:], Act.Sin)

                    g = gts[t]
                    u = trigp.tile([128, nchunk, N], f32, tag="u", name="u")
                    # Square is in every ACT table set: no table reload.
                    nc.scalar.activation(u[:], t2[:], Act.Square,
                                         scale=float(np.sqrt(2.0)))
                    p1 = pp.tile([128, nchunk, N], f32, tag="p1", name="p1")
                    nc.vector.scalar_tensor_tensor(
                        p1[:], u[:], 1.0, g[:], Alu.subtract, Alu.mult)
                    p2 = pp.tile([128, nchunk, N], f32, tag="p2", name="p2")
                    nc.vector.tensor_mul(p2[:], g[:], ss[:])

                    if t % 2 == 0:
                        po = accp.tile([3, 2, TR, TC], f32, tag="po", name="po")
                    chunks = [(p1, c) for c in range(nchunk)] + \
                             [(p2, c) for c in range(nchunk)]
                    for ci, (src, c) in enumerate(chunks):
                        nc.tensor.matmul(
                            po[:, t % 2],
                            ab_sb[:, ci * 3:(ci + 1) * 3],
                            src[:, c, :],
                            start=(ci == 0), stop=(ci == len(chunks) - 1),
                        )
                    if t % 2 == 1:
                        ob = pp.tile([3, 2, TR, TC], f32, tag="ob", name="ob")
                        nc.scalar.copy(ob[:], po[:])
                        for h in range(2):
                            ti = t0 + t - 1 + h
                            trow, tcol = divmod(ti, TPR)
                            nc.sync.dma_start(
                                out=outd[:, trow * TR:(trow + 1) * TR,
                                         tcol * TC:(tcol + 1) * TC],
                                in_=ob[:, h],
                            )

    nc.compile()
    return nc


def _wrap(x):
    return np.mod(x + np.pi, 2.0 * np.pi) - np.pi


def _host_arrays(inputs):
    """Fold parameters, cull gabors per core, build device arrays."""
    gx = np.asarray(inputs["grid_x"], np.float64)
    gy = np.asarray(inputs["grid_y"], np.float64)
    u = np.clip(np.asarray(inputs["u"], np.float64), -1, 1)
    v = np.clip(np.asarray(inputs["v"], np.float64), -1, 1)
    th = np.clip(np.asarray(inputs["theta"], np.float64), -2, 2) * (2 * np.pi)
    sig = np.clip(np.asarray(inputs["rel_sigma"], np.float64), 0.001, 1.0)
    rf = np.clip(np.asarray(inputs["rel_freq"], np.float64), -5, 5)
    gam = np.clip(np.asarray(inputs["gamma"], np.float64), 0.0001, 1.0)
    psi = np.clip(np.asarray(inputs["psi"], np.float64), -1, 1)
    amp = np.clip(np.asarray(inputs["amplitude"], np.float64), 0, 1)

    cr, sr = np.cos(th), np.sin(th)
    cx = -(cr * u + sr * v)       # x_rot = cr*X + sr*Y + cx
    cy = sr * u - cr * v          # y_rot = -sr*X + cr*Y + cy
    p = 1.0 / (2.0 * sig * sig)
    q = 1.0 / (2.0 * gam * gam)
    freq = 2 * np.pi / np.exp(rf)
    phase = psi * (2 * np.pi)                     # [G,3]
    alpha = amp * np.cos(phase)                   # [G,3]
    beta = -amp * np.sin(phase)

    # --- cull gabors per core: the reference only evaluates at pixel
    # positions, so the keep test is the exact per-pixel max of E over the
    # strip (full resolution -- gamma can be 1e-4, a ridge 0.05 px wide).
    ampmax = amp.max(1)
    elim = np.log(np.maximum(CULL_THR / np.maximum(ampmax, 1e-30), 1e-300)) - 1.0
    keep_lists = []
    crf = cr.astype(np.float32)[:, None]
    srf = sr.astype(np.float32)[:, None]
    pf = p.astype(np.float32)[:, None]
    qf = q.astype(np.float32)[:, None]
    for core in range(NCORES):
        Xs = np.asarray(gx[core * SH:(core + 1) * SH], np.float32).ravel()[None, :]
        Ys = np.asarray(gy[core * SH:(core + 1) * SH], np.float32).ravel()[None, :]
        dx = Xs - u.astype(np.float32)[:, None]
        dy = Ys - v.astype(np.float32)[:, None]
        xr = dx * crf + dy * srf
        yr = dy * crf - dx * srf
        quad = xr * xr * pf
        quad += yr * yr * qf
        Em = -quad.min(1)
        keep = np.flatnonzero(Em >= elim)
        keep_lists.append(keep)
    gmax = max(len(k) for k in keep_lists)
    nchunk = max(1, -(-gmax // 128))
    Gc = 128 * nchunk

    # Tile grids: [total_tiles, N] with strip-row-major tile order.
    Xt = gx.reshape(H // TR, TR, W // TC, TC).transpose(0, 2, 1, 3).reshape(-1, N)
    Yt = gy.reshape(H // TR, TR, W // TC, TC).transpose(0, 2, 1, 3).reshape(-1, N)
    Xc = Xt.mean(1)
    Yc = Yt.mean(1)
    dxf = Xt - Xc[:, None]
    dyf = Yt - Yc[:, None]
    feat = np.stack([dxf, dyf, np.ones_like(dxf), dxf * dxf, dyf * dyf,
                     dxf * dyf], 0)             # [6, T, N]

    # Static one-hot features (bf16-exact).
    onehot = np.zeros((KS, N), np.float32)
    ii, jj = np.divmod(np.arange(N), TC)
    onehot[ii, np.arange(N)] = 1.0
    onehot[TR + jj, np.arange(N)] = 1.0

    yrow_all = Yt.reshape(-1, TR, TC)[:, :, 0]   # [T, TR]
    xcol_all = Xt.reshape(-1, TR, TC)[:, 0, :]   # [T, TC]

    in_maps = []
    for core in range(NCORES):
        keep = keep_lists[core]
        k = len(keep)
        sl = slice(core * NT, (core + 1) * NT)
        crk, srk = cr[keep], sr[keep]
        cxk, cyk = cx[keep], cy[keep]
        pk, qk = p[keep], q[keep]
        fk = freq[keep]

        XcT = Xc[sl][:, None]
        YcT = Yc[sl][:, None]
        cxt = XcT * crk[None, :] + YcT * srk[None, :] + cxk[None, :]  # [NT,k]
        cyt = -XcT * srk[None, :] + YcT * crk[None, :] + cyk[None, :]
        WE = np.zeros((6, NT, Gc), np.float32)
        WE[0, :, :k] = -(2 * pk * crk * cxt - 2 * qk * srk * cyt)
        WE[1, :, :k] = -(2 * pk * srk * cxt + 2 * qk * crk * cyt)
        WE[2, :, :k] = -(pk * cxt * cxt + qk * cyt * cyt)
        WE[3, :, :k] = -(pk * crk * crk + qk * srk * srk)
        WE[4, :, :k] = -(pk * srk * srk + qk * crk * crk)
        WE[5, :, :k] = -(2 * pk * crk * srk - 2 * qk * srk * crk)

        yrow = yrow_all[sl]                              # [NT, TR]
        xcol = xcol_all[sl]                              # [NT, TC]
        A = _wrap(fk[None, :, None] * srk[None, :, None]
                  * (yrow[:, None, :] - YcT[:, :, None]))            # [NT,k,TR]
        Bt = _wrap(fk[None, :, None] * crk[None, :, None]
                   * (xcol[:, None, :] - XcT[:, :, None])
                   + (fk[None, :] * cxt)[:, :, None])                # [NT,k,TC]
        WS = np.zeros((NT, Gc, KS), np.float32)
        WS[:, :k, :TR] = A
        WS[:, :k, TR:] = Bt
        WSh = _to_bf16(WS).astype(np.float32)
        WSl = WS - WSh
        WSh = WSh.transpose(2, 0, 1)                     # [KS, NT, Gc]
        WSl = np.ascontiguousarray(WSl.transpose(2, 0, 1), dtype=np.float32)

        AB = np.zeros((128, nchunk * 2 * 3), np.float32)
        al = np.zeros((Gc, 3)); bt = np.zeros((Gc, 3))
        al[:k] = alpha[keep]
        bt[:k] = beta[keep]
        for c in range(nchunk):
            AB[:, 3 * c:3 * c + 3] = -al[c * 128:(c + 1) * 128]
            off = 3 * (nchunk + c)
            AB[:, off:off + 3] = bt[c * 128:(c + 1) * 128]

        in_maps.append({
            "feat": np.ascontiguousarray(feat[:, sl, :], dtype=np.float32),
            "we": np.ascontiguousarray(WE, dtype=np.float32),
            "wsh": _to_bf16(WSh),
            "wsl": _to_bf16(WSl),
            "onehot": _to_bf16(onehot),
            "ab": AB,
        })
    return in_maps, nchunk


def _to_bf16(a):
    import ml_dtypes
    return np.ascontiguousarray(a.astype(ml_dtypes.bfloat16))


def _get_program(nchunk):
    if nchunk not in _PROGRAMS:
        _PROGRAMS[nchunk] = _build_program(nchunk)
    return _PROGRAMS[nchunk]


def kernel(**inputs):
    from concourse.bass_utils import run_bass_kernel_spmd

    in_maps, nchunk = _host_arrays(inputs)
    nc = _get_program(nchunk)
    res = run_bass_kernel_spmd(nc, in_maps, list(range(NCORES)))
    out = np.empty((3, H, W), np.float32)
    for core in range(NCORES):
        out[:, core * SH:(core + 1) * SH, :] = res.results[core]["out"]
    np.clip(out, -1.0, 1.0, out=out)
    return out



# revision 6
# speedup vs baseline: 1.3011x; 1.3011x over previous
"""Gabor layer Trainium2 kernel (v2).

Per gabor g and pixel (x,y): amp[g,c] * exp(E) * cos(S + phase[g,c]).
cos(S+p) = cos(p)cos(S) - sin(p)sin(S) turns the channel sum over g into
matmuls over gauss*cos(S) / gauss*sin(S) planes (contraction = gabors).

v2 speedups over the fp32 baseline (276us -> target ~115us):
  * E matmul: integer tile-local features [dj,di,1,dj2,di2,djdi] (exact in
    f32r) x hi/lo-split f32r weights, K-stacked into ONE 1-cycle/row f32r
    matmul (fp32 runs at 4 cycles/row; f32r at 1 for N>=256, and the PE
    f32r product is exact for pre-rounded inputs - measured 1.5e-7).
    Features are tile-independent -> single resident [12,512] tensor.
  * S matmul: bf16 hi/lo tables K-stacked with a duplicated one-hot into
    ONE K=96 bf16 matmul per tile (cost scales with N only).
  * sin+cos: w1 = wrap(S), w2 = wrap(S+pi/2) (DVE add_range_wrap), then
    two Sin activations; replaces Sin/Square/Sin chain (ACT was 80% busy).
  * products p1 = cos*gauss, p2 = sin*gauss in fp16 (DVE 2x 16-bit mode),
    feeding fp16 output matmuls (1 cycle/row vs 4 for fp32).
  * output: DMA straight from PSUM to DRAM (drops the ACT copy),
    tile-major DRAM layout, host reassembles strips.
  * B=16 tile blocks halve ACT table swaps (Exp/Sin live in different
    hardware table sets; two loads per block).

Sharding: 8 cores x 64-row strips, no collectives; final clamp and strip
concatenation on host. Gabors culled per strip (exact per-pixel E max
test), padded to 128-chunks.
"""

import os
import sys

import numpy as np

for _p in ("/opt/trn_rl_repo",):
    if os.path.isdir(_p) and _p not in sys.path:
        sys.path.append(_p)

H = W = 512
G = 256
NCORES = 8
SH = H // NCORES      # strip rows per core
TR, TC = 16, 32       # tile rows x cols
N = TR * TC           # 512 pixels per tile
TPR = W // TC         # tiles per strip row = 16
NT = (SH // TR) * TPR # tiles per core = 64
B = 8                 # tiles per block (ACT table phase granularity)
NBLK = NT // B
KS = 2 * (TR + TC)    # one-hot rows: [rowhi, colhi, rowlo, collo] = 96
PI = float(np.pi)
CULL_THR = 1e-7

_PROGRAMS = {}


def _build_program(nchunk):
    from concourse import bacc, mybir, tile

    f32 = mybir.dt.float32
    f32r = mybir.dt.float32r
    bf16 = mybir.dt.bfloat16
    f16 = mybir.dt.float16
    Act = mybir.ActivationFunctionType
    Gc = 128 * nchunk
    mmbufs = 2 if nchunk == 1 else 1

    nc = bacc.Bacc("TRN2", target_bir_lowering=False, debug=False,
                   num_devices=NCORES)

    featd = nc.dram_tensor("feat", [12, N], f32r, kind="ExternalInput")
    ohd = nc.dram_tensor("onehot", [KS, N], bf16, kind="ExternalInput")
    wed = nc.dram_tensor("we", [12, NT, Gc], f32r, kind="ExternalInput")
    wsd = nc.dram_tensor("ws", [KS, NT, Gc], bf16, kind="ExternalInput")
    abd = nc.dram_tensor("ab", [128, nchunk * 2 * 3], f16, kind="ExternalInput")
    outd = nc.dram_tensor("out", [3, NT, N], f32, kind="ExternalOutput")

    with tile.TileContext(nc) as tc:
        with (
            tc.tile_pool(name="io", bufs=1) as iop,
            tc.tile_pool(name="gauss", bufs=B // 2 + 2) as gp,
            tc.tile_pool(name="trig", bufs=3) as trigp,
            tc.tile_pool(name="prod", bufs=3) as pp,
            tc.tile_pool(name="mme", bufs=mmbufs, space="PSUM") as mmep,
            tc.tile_pool(name="mms", bufs=mmbufs, space="PSUM") as mmsp,
            tc.tile_pool(name="acc", bufs=2, space="PSUM") as accp,
        ):
            ab_sb = iop.tile([128, nchunk * 2 * 3], f16, tag="ab")
            nc.sync.dma_start(out=ab_sb[:], in_=abd[:])
            oh_sb = iop.tile([KS, N], bf16, tag="oh")
            nc.sync.dma_start(out=oh_sb[:], in_=ohd[:])
            ft_sb = iop.tile([12, N], f32r, tag="ft")
            nc.sync.dma_start(out=ft_sb[:], in_=featd[:])

            for blk in range(NBLK):
                t0 = blk * B
                we = iop.tile([12, B, Gc], f32r, tag="we", bufs=2)
                nc.sync.dma_start(out=we[:], in_=wed[:, t0:t0 + B, :])
                ws = iop.tile([KS, B, Gc], bf16, tag="ws", bufs=2)
                nc.sync.dma_start(out=ws[:], in_=wsd[:, t0:t0 + B, :])

                # Phase A: gaussians for the block (Exp table loaded).
                gts = []
                for t in range(B):
                    mE = mmep.tile([128, nchunk, N], f32, tag="mE", name="mE")
                    for c in range(nchunk):
                        nc.tensor.matmul(
                            mE[:, c, :],
                            we[:, t, c * 128:(c + 1) * 128],
                            ft_sb[:],
                            start=True, stop=True,
                        )
                    if t % 2 == 0:
                        gpair = gp.tile([128, 2 * nchunk, N], f16, tag="g",
                                        name="gauss")
                        gts.append(gpair)
                    nc.scalar.activation(
                        gpair[:, (t % 2) * nchunk:(t % 2 + 1) * nchunk],
                        mE[:], Act.Exp)

                # Phase B: sinusoid + products + reduction (Sin table).
                for t in range(B):
                    mS = mmsp.tile([128, nchunk, N], f32, tag="mS", name="mS")
                    for c in range(nchunk):
                        nc.tensor.matmul(
                            mS[:, c, :],
                            ws[:, t, c * 128:(c + 1) * 128],
                            oh_sb[:],
                            start=True, stop=True,
                        )
                    if t % 2 == 0:
                        w1p = trigp.tile([128, 2 * nchunk, N], f16, tag="w1",
                                         name="w1")
                    nc.vector.add_range_wrap(
                        w1p[:, (t % 2) * nchunk:(t % 2 + 1) * nchunk],
                        mS[:], 0.0, PI, 2.0 * PI)
                    if t % 2 == 1:
                        w2p = trigp.tile([128, 2 * nchunk, N], f16, tag="w2",
                                         name="w2")
                        nc.vector.add_range_wrap(w2p[:], w1p[:],
                                                 PI / 2, PI, 2.0 * PI)
                        ssp = trigp.tile([128, 2 * nchunk, N], f16, tag="ss",
                                         name="ss")
                        nc.scalar.activation(ssp[:], w1p[:], Act.Sin)
                        csp = trigp.tile([128, 2 * nchunk, N], f16, tag="cs",
                                         name="cs")
                        nc.scalar.activation(csp[:], w2p[:], Act.Sin)

                        gpair = gts[t // 2]
                        p1p = pp.tile([128, 2 * nchunk, N], f16, tag="p1",
                                      name="p1")
                        nc.vector.tensor_mul(p1p[:], gpair[:], csp[:])
                        p2p = pp.tile([128, 2 * nchunk, N], f16, tag="p2",
                                      name="p2")
                        nc.vector.tensor_mul(p2p[:], gpair[:], ssp[:])

                        po = accp.tile([3, 2, N], f32, tag="po", name="po")
                        for h in range(2):
                            ops = [(p1p, c) for c in range(nchunk)] + \
                                  [(p2p, c) for c in range(nchunk)]
                            for ci, (src, c) in enumerate(ops):
                                ab_col = (0 if src is p1p else 3 * nchunk) + 3 * c
                                nc.tensor.matmul(
                                    po[:, h],
                                    ab_sb[:, ab_col:ab_col + 3],
                                    src[:, h * nchunk + c, :],
                                    start=(ci == 0), stop=(ci == len(ops) - 1),
                                )
                        ob = pp.tile([3, 2, N], f32, tag="ob", name="ob")
                        nc.vector.tensor_copy(ob[:], po[:])
                        nc.sync.dma_start(
                            out=outd[:, t0 + t - 1:t0 + t + 1, :],
                            in_=ob[:],
                        )

    nc.compile()
    return nc


def _wrap(x):
    return np.mod(x + np.pi, 2.0 * np.pi) - np.pi


def _to_f32r(a):
    b = np.ascontiguousarray(a, np.float32).view(np.uint32)
    r = (b + np.uint32(0x7FF) + ((b >> np.uint32(12)) & np.uint32(1))) \
        & np.uint32(0xFFFFF000)
    return r.view(np.float32)


def _to_bf16(a):
    import ml_dtypes
    return np.ascontiguousarray(a.astype(ml_dtypes.bfloat16))


def _host_arrays(inputs):
    """Fold parameters, cull gabors per core, build device arrays."""
    gx = np.asarray(inputs["grid_x"], np.float64)
    gy = np.asarray(inputs["grid_y"], np.float64)
    u = np.clip(np.asarray(inputs["u"], np.float64), -1, 1)
    v = np.clip(np.asarray(inputs["v"], np.float64), -1, 1)
    th = np.clip(np.asarray(inputs["theta"], np.float64), -2, 2) * (2 * np.pi)
    sig = np.clip(np.asarray(inputs["rel_sigma"], np.float64), 0.001, 1.0)
    rf = np.clip(np.asarray(inputs["rel_freq"], np.float64), -5, 5)
    gam = np.clip(np.asarray(inputs["gamma"], np.float64), 0.0001, 1.0)
    psi = np.clip(np.asarray(inputs["psi"], np.float64), -1, 1)
    amp = np.clip(np.asarray(inputs["amplitude"], np.float64), 0, 1)

    cr, sr = np.cos(th), np.sin(th)
    cx = -(cr * u + sr * v)       # x_rot = cr*X + sr*Y + cx
    cy = sr * u - cr * v          # y_rot = -sr*X + cr*Y + cy
    p = 1.0 / (2.0 * sig * sig)
    q = 1.0 / (2.0 * gam * gam)
    freq = 2 * np.pi / np.exp(rf)
    phase = psi * (2 * np.pi)                     # [G,3]
    alpha = amp * np.cos(phase)                   # [G,3]
    beta = -amp * np.sin(phase)

    # --- cull gabors per core (exact per-pixel max of E over the strip).
    ampmax = amp.max(1)
    elim = np.log(np.maximum(CULL_THR / np.maximum(ampmax, 1e-30), 1e-300)) - 1.0
    keep_lists = []
    crf = cr.astype(np.float32)[:, None]
    srf = sr.astype(np.float32)[:, None]
    pf = p.astype(np.float32)[:, None]
    qf = q.astype(np.float32)[:, None]
    for core in range(NCORES):
        Xs = np.asarray(gx[core * SH:(core + 1) * SH], np.float32).ravel()[None, :]
        Ys = np.asarray(gy[core * SH:(core + 1) * SH], np.float32).ravel()[None, :]
        dx = Xs - u.astype(np.float32)[:, None]
        dy = Ys - v.astype(np.float32)[:, None]
        xr = dx * crf + dy * srf
        yr = dy * crf - dx * srf
        quad = xr * xr * pf
        quad += yr * yr * qf
        Em = -quad.min(1)
        keep = np.flatnonzero(Em >= elim)
        keep_lists.append(keep)
    gmax = max(len(k) for k in keep_lists)
    nchunk = max(1, -(-gmax // 128))
    Gc = 128 * nchunk

    # Tile grids (tile-major order): [total_tiles, N]
    Xt = gx.reshape(H // TR, TR, W // TC, TC).transpose(0, 2, 1, 3).reshape(-1, N)
    Yt = gy.reshape(H // TR, TR, W // TC, TC).transpose(0, 2, 1, 3).reshape(-1, N)

    # Integer tile-local offsets (identical for every tile): pixel (i,j)
    # in-tile -> di = i-8, dj = j-16. The grid is affine in the index
    # (linspace), so x = xc + dj*hx with hx the column step; hx/xc are
    # recovered from the actual grid rows (works for any affine grid).
    ii, jj = np.divmod(np.arange(N), TC)
    di = (ii - TR // 2).astype(np.float64)
    dj = (jj - TC // 2).astype(np.float64)
    feat6 = np.stack([dj, di, np.ones_like(dj), dj * dj, di * di, dj * di], 0)
    feat12 = np.concatenate([feat6, feat6], 0).astype(np.float32)  # exact ints

    hx = (Xt[:, 1] - Xt[:, 0])                    # [T] column step
    hy = (Yt[:, TC] - Yt[:, 0])                   # [T] row step
    Xc = Xt[:, TR // 2 * TC + TC // 2]            # x at (di=0, dj=0)
    Yc = Yt[:, TR // 2 * TC + TC // 2]

    # One-hot features: [row(16) hi, col(32) hi, row lo, col lo]
    onehot = np.zeros((KS, N), np.float32)
    onehot[ii, np.arange(N)] = 1.0
    onehot[TR + jj, np.arange(N)] = 1.0
    onehot[TR + TC:] = onehot[:TR + TC]

    yrow_all = Yt.reshape(-1, TR, TC)[:, :, 0]   # [T, TR]
    xcol_all = Xt.reshape(-1, TR, TC)[:, 0, :]   # [T, TC]

    in_maps = []
    for core in range(NCORES):
        keep = keep_lists[core]
        k = len(keep)
        sl = slice(core * NT, (core + 1) * NT)
        crk, srk = cr[keep], sr[keep]
        cxk, cyk = cx[keep], cy[keep]
        pk, qk = p[keep], q[keep]
        fk = freq[keep]

        XcT = Xc[sl][:, None]                     # [NT, 1]
        YcT = Yc[sl][:, None]
        hxT = hx[sl][:, None]
        hyT = hy[sl][:, None]
        cxt = XcT * crk[None, :] + YcT * srk[None, :] + cxk[None, :]  # [NT,k]
        cyt = -XcT * srk[None, :] + YcT * crk[None, :] + cyk[None, :]
        # E in integer-offset features: x_rot = cxt + hx*cr*dj + hy*sr*di
        #                               y_rot = cyt - hx*sr*dj + hy*cr*di
        a1 = hxT * crk[None, :]   # dj coef in x_rot
        a2 = hyT * srk[None, :]   # di coef in x_rot
        b1 = -hxT * srk[None, :]  # dj coef in y_rot
        b2 = hyT * crk[None, :]   # di coef in y_rot
        WE = np.zeros((6, NT, Gc))
        WE[0, :, :k] = -2.0 * (pk * cxt * a1 + qk * cyt * b1)        # dj
        WE[1, :, :k] = -2.0 * (pk * cxt * a2 + qk * cyt * b2)        # di
        WE[2, :, :k] = -(pk * cxt * cxt + qk * cyt * cyt)            # 1
        WE[3, :, :k] = -(pk * a1 * a1 + qk * b1 * b1)                # dj^2
        WE[4, :, :k] = -(pk * a2 * a2 + qk * b2 * b2)                # di^2
        WE[5, :, :k] = -2.0 * (pk * a1 * a2 + qk * b1 * b2)          # dj*di
        WEh = _to_f32r(WE)
        WEl = _to_f32r(WE - WEh)
        we12 = np.concatenate([WEh, WEl], 0)      # [12, NT, Gc] f32r bits

        yrow = yrow_all[sl]                       # [NT, TR]
        xcol = xcol_all[sl]                       # [NT, TC]
        A = _wrap(fk[None, :, None] * srk[None, :, None]
                  * (yrow[:, None, :] - YcT[:, :, None]))            # [NT,k,TR]
        Bt = _wrap(fk[None, :, None] * crk[None, :, None]
                   * (xcol[:, None, :] - XcT[:, :, None])
                   + (fk[None, :] * cxt)[:, :, None])                # [NT,k,TC]
        WS = np.zeros((NT, Gc, TR + TC))
        WS[:, :k, :TR] = A
        WS[:, :k, TR:] = Bt
        WSh = _to_bf16(WS).astype(np.float64)
        WSl = WS - WSh
        ws = np.concatenate(
            [WSh.transpose(2, 0, 1), WSl.transpose(2, 0, 1)], 0
        )                                         # [96, NT, Gc]

        AB = np.zeros((128, nchunk * 2 * 3), np.float64)
        al = np.zeros((Gc, 3)); bt = np.zeros((Gc, 3))
        al[:k] = alpha[keep]
        bt[:k] = beta[keep]
        for c in range(nchunk):
            AB[:, 3 * c:3 * c + 3] = al[c * 128:(c + 1) * 128]
            off = 3 * (nchunk + c)
            AB[:, off:off + 3] = bt[c * 128:(c + 1) * 128]

        in_maps.append({
            "feat": feat12,
            "onehot": _to_bf16(onehot),
            "we": np.ascontiguousarray(we12, np.float32),
            "ws": _to_bf16(ws),
            "ab": AB.astype(np.float16),
        })
    return in_maps, nchunk


def _get_program(nchunk):
    if nchunk not in _PROGRAMS:
        _PROGRAMS[nchunk] = _build_program(nchunk)
    return _PROGRAMS[nchunk]


def kernel(**inputs):
    from concourse.bass_utils import run_bass_kernel_spmd

    in_maps, nchunk = _host_arrays(inputs)
    nc = _get_program(nchunk)
    res = run_bass_kernel_spmd(nc, in_maps, list(range(NCORES)))
    out = np.empty((3, H, W), np.float32)
    for core in range(NCORES):
        r = res.results[core]["out"]              # [3, NT, N] tile-major
        out[:, core * SH:(core + 1) * SH, :] = (
            r.reshape(3, SH // TR, TPR, TR, TC)
             .transpose(0, 1, 3, 2, 4)
             .reshape(3, SH, W)
        )
    np.clip(out, -1.0, 1.0, out=out)
    return out


# revision 11
# speedup vs baseline: 2.3617x; 1.8151x over previous
"""Gabor layer Trainium2 kernel (v3: packed planes).

Per gabor g and pixel (x,y): amp[g,c] * exp(E) * cos(S + phase[g,c]).
cos(S+p) = cos(p)cos(S) - sin(p)sin(S) turns the channel sum over g into
matmuls over gauss*cos(S) / gauss*sin(S) planes (contraction = gabors).

All elementwise engine costs scale with the free (pixel) axis only, so the
partition axis is free parallelism. Each 64-row strip is culled per column
half; with kL,kR <= 64 two tiles (one left-half, one right-half) pack into
one 128-partition plane, halving every exp/sin/wrap/multiply:
  plane pl = (row_block, col_block<8): partitions 0:64 carry the left
  tile's gabors, 64:128 the right tile's (tile tR = tL + 8).

Per plane:  E = [WEh;WEl]^T @ feat12     (1 f32r matmul; integer tile-local
            features [dj,di,1,dj2,di2,dj*di] are exact in f32r, weights
            hi/lo split; f32r runs 1 cycle/row vs 4 for fp32 and the PE
            product is exact for pre-rounded inputs)
            S = [Ah;Bh;Al;Bl]^T @ onehot96   (1 bf16 matmul, K=96)
            gauss = Exp(E) fp16; w1 = wrap(S); w2 = wrap(S+pi/2) (DVE);
            ss,cs = Sin(w1),Sin(w2) fp16; p1 = cs*gauss, p2 = ss*gauss
            (fp16 DVE 2x mode)
Output: 4 logical tiles accumulate into ONE PSUM bank at partition offsets
0/32/64/96 (tile_position column tiling, one accumulation group per bank),
then one 512-cycle DVE copy + 2 DMAs per quad.

Two global phases (all Exps, then all Sins) keep the Exp/Sin activation
tables from thrashing: they live in different hardware table sets and each
swap costs 1.3us (the v2 interleaved phasing measured 36 loads = 46us).

Sharding: 8 cores x 64-row strips, no collectives; clamp + reassembly on
host. Falls back to the v2 per-tile program if a column half keeps > 64
gabors (not the case for the reference inputs: kL<=55, kR<=64).
"""

import os
import sys

import numpy as np

for _p in ("/opt/trn_rl_repo",):
    if os.path.isdir(_p) and _p not in sys.path:
        sys.path.append(_p)

H = W = 512
G = 256
NCORES = 8
SH = H // NCORES      # strip rows per core
TR, TC = 16, 32       # tile rows x cols
N = TR * TC           # 512 pixels per tile
TPR = W // TC         # tiles per strip row = 16
NT = (SH // TR) * TPR # tiles per core = 64
NPL = NT // 2         # packed planes per core = 32
KS = 2 * (TR + TC)    # one-hot rows: [rowhi, colhi, rowlo, collo] = 96
PI = float(np.pi)
CULL_THR = 1e-7

_PROGRAMS = {}


def _build_program_packed():
    from concourse import bacc, mybir, tile

    f32 = mybir.dt.float32
    f32r = mybir.dt.float32r
    bf16 = mybir.dt.bfloat16
    f16 = mybir.dt.float16
    Act = mybir.ActivationFunctionType

    nc = bacc.Bacc("TRN2", target_bir_lowering=False, debug=False,
                   num_devices=NCORES)

    featd = nc.dram_tensor("feat", [12, N], f32r, kind="ExternalInput")
    ohd = nc.dram_tensor("onehot", [KS, N], bf16, kind="ExternalInput")
    wed = nc.dram_tensor("we", [12, NPL, 128], f32r, kind="ExternalInput")
    wsd = nc.dram_tensor("ws", [KS, NPL, 128], bf16, kind="ExternalInput")
    abd = nc.dram_tensor("ab", [128, 6], f16, kind="ExternalInput")
    outd = nc.dram_tensor("out", [2, 3, NPL, N], f32, kind="ExternalOutput")

    with tile.TileContext(nc) as tc:
        with (
            tc.tile_pool(name="io", bufs=1) as iop,
            tc.tile_pool(name="gauss", bufs=NPL // 2 + 1) as gp,
            tc.tile_pool(name="trig", bufs=3) as trigp,
            tc.tile_pool(name="prod", bufs=3) as pp,
            tc.tile_pool(name="mm", bufs=2, space="PSUM") as mmp,
            tc.tile_pool(name="acc", bufs=2, space="PSUM") as accp,
        ):
            ab_sb = iop.tile([128, 6], f16, tag="ab")
            nc.sync.dma_start(out=ab_sb[:], in_=abd[:])
            oh_sb = iop.tile([KS, N], bf16, tag="oh")
            nc.sync.dma_start(out=oh_sb[:], in_=ohd[:])
            ft_sb = iop.tile([12, N], f32r, tag="ft")
            nc.sync.dma_start(out=ft_sb[:], in_=featd[:])
            we = iop.tile([12, NPL, 128], f32r, tag="we")
            nc.sync.dma_start(out=we[:], in_=wed[:])
            ws = iop.tile([KS, NPL, 128], bf16, tag="ws")
            nc.sync.dma_start(out=ws[:], in_=wsd[:])

            # Phase A: all gaussians (Exp table stays loaded).
            gts = []
            for q in range(NPL // 2):
                mEp = mmp.tile([128, 2, N], f32, tag="mm", name="mE")
                for h in range(2):
                    nc.tensor.matmul(
                        mEp[:, h, :],
                        we[:, 2 * q + h, :],
                        ft_sb[:],
                        start=True, stop=True,
                    )
                gpair = gp.tile([128, 2, N], f16, tag="g", name="gauss")
                gts.append(gpair)
                nc.scalar.activation(gpair[:], mEp[:], Act.Exp)

            # Phase B: sinusoids + products + reduction (Sin table).
            for q in range(NPL // 2):
                mSp = mmp.tile([128, 2, N], f32, tag="mm", name="mS")
                for h in range(2):
                    nc.tensor.matmul(
                        mSp[:, h, :],
                        ws[:, 2 * q + h, :],
                        oh_sb[:],
                        start=True, stop=True,
                    )
                w1p = trigp.tile([128, 2, N], f16, tag="w1", name="w1")
                nc.vector.add_range_wrap(w1p[:], mSp[:], 0.0, PI, 2.0 * PI)
                w2p = trigp.tile([128, 2, N], f16, tag="w2", name="w2")
                nc.vector.add_range_wrap(w2p[:], w1p[:], PI / 2, PI, 2.0 * PI)
                ssp = trigp.tile([128, 2, N], f16, tag="ss", name="ss")
                nc.scalar.activation(ssp[:], w1p[:], Act.Sin)
                csp = trigp.tile([128, 2, N], f16, tag="cs", name="cs")
                nc.scalar.activation(csp[:], w2p[:], Act.Sin)

                gpair = gts[q]
                p1p = pp.tile([128, 2, N], f16, tag="p1", name="p1")
                nc.vector.tensor_mul(p1p[:], gpair[:], csp[:])
                p2p = pp.tile([128, 2, N], f16, tag="p2", name="p2")
                nc.vector.tensor_mul(p2p[:], gpair[:], ssp[:])

                # 2 logical tiles per PSUM bank at partition offsets 0/32
                # (base 96 is illegal - PE quadrant 3), bank h = plane h of
                # the pair; one accumulation group per bank (the first
                # matmul's bank-clear covers both regions).
                po = accp.tile([128, 2, N], f32, tag="po", name="po")
                for h in range(2):        # plane within pair = bank
                    for s in range(2):    # side: 0=left(K 0:64) 1=right
                        ks, co = s * 64, s * 32
                        for pi_, (src, acol) in enumerate(
                            ((p1p, 0), (p2p, 3))
                        ):
                            nc.tensor.matmul(
                                po[co:co + 3, h, :],
                                ab_sb[ks:ks + 64, acol:acol + 3],
                                src[ks:ks + 64, h, :],
                                start=(pi_ == 0), stop=(pi_ == 1),
                                skip_group_check=True,
                            )
                ob = pp.tile([128, 2, N], f32, tag="ob", name="ob")
                nc.vector.tensor_copy(ob[:], po[:])
                nc.sync.dma_start(out=outd[0, :, 2 * q:2 * q + 2, :],
                                  in_=ob[0:3, :, :])
                nc.sync.dma_start(out=outd[1, :, 2 * q:2 * q + 2, :],
                                  in_=ob[32:35, :, :])

    nc.compile()
    return nc


def _wrap(x):
    return np.mod(x + np.pi, 2.0 * np.pi) - np.pi


def _to_f32r(a):
    b = np.ascontiguousarray(a, np.float32).view(np.uint32)
    r = (b + np.uint32(0x7FF) + ((b >> np.uint32(12)) & np.uint32(1))) \
        & np.uint32(0xFFFFF000)
    return r.view(np.float32)


def _to_bf16(a):
    import ml_dtypes
    return np.ascontiguousarray(a.astype(ml_dtypes.bfloat16))


def _fold_params(inputs):
    u = np.clip(np.asarray(inputs["u"], np.float64), -1, 1)
    v = np.clip(np.asarray(inputs["v"], np.float64), -1, 1)
    th = np.clip(np.asarray(inputs["theta"], np.float64), -2, 2) * (2 * np.pi)
    sig = np.clip(np.asarray(inputs["rel_sigma"], np.float64), 0.001, 1.0)
    rf = np.clip(np.asarray(inputs["rel_freq"], np.float64), -5, 5)
    gam = np.clip(np.asarray(inputs["gamma"], np.float64), 0.0001, 1.0)
    psi = np.clip(np.asarray(inputs["psi"], np.float64), -1, 1)
    amp = np.clip(np.asarray(inputs["amplitude"], np.float64), 0, 1)
    cr, sr = np.cos(th), np.sin(th)
    return dict(
        u=u, v=v, cr=cr, sr=sr,
        cx=-(cr * u + sr * v), cy=sr * u - cr * v,
        p=1.0 / (2.0 * sig * sig), q=1.0 / (2.0 * gam * gam),
        freq=2 * np.pi / np.exp(rf),
        alpha=amp * np.cos(psi * 2 * np.pi),
        beta=-amp * np.sin(psi * 2 * np.pi),
        amp=amp,
    )


def _keeps(P, gx, gy, rows, cols):
    """Exact per-pixel cull: keep gabors whose max E over the region
    clears the contribution threshold."""
    ampmax = P["amp"].max(1)
    elim = np.log(np.maximum(CULL_THR / np.maximum(ampmax, 1e-30),
                             1e-300)) - 1.0
    crf = P["cr"].astype(np.float32)[:, None]
    srf = P["sr"].astype(np.float32)[:, None]
    pf = P["p"].astype(np.float32)[:, None]
    qf = P["q"].astype(np.float32)[:, None]
    Xs = np.asarray(gx[rows][:, cols], np.float32).ravel()[None, :]
    Ys = np.asarray(gy[rows][:, cols], np.float32).ravel()[None, :]
    dx = Xs - P["u"].astype(np.float32)[:, None]
    dy = Ys - P["v"].astype(np.float32)[:, None]
    xr = dx * crf + dy * srf
    yr = dy * crf - dx * srf
    quad = xr * xr * pf
    quad += yr * yr * qf
    Em = -quad.min(1)
    return np.flatnonzero(Em >= elim)


def _tile_geometry(gx, gy):
    """Tile-major grids and per-tile affine centers/steps."""
    Xt = gx.reshape(H // TR, TR, W // TC, TC).transpose(0, 2, 1, 3).reshape(-1, N)
    Yt = gy.reshape(H // TR, TR, W // TC, TC).transpose(0, 2, 1, 3).reshape(-1, N)
    hx = Xt[:, 1] - Xt[:, 0]
    hy = Yt[:, TC] - Yt[:, 0]
    Xc = Xt[:, TR // 2 * TC + TC // 2]
    Yc = Yt[:, TR // 2 * TC + TC // 2]
    yrow = Yt.reshape(-1, TR, TC)[:, :, 0]
    xcol = Xt.reshape(-1, TR, TC)[:, 0, :]
    return Xc, Yc, hx, hy, yrow, xcol


def _tile_tables(P, keep, tiles, Xc, Yc, hx, hy, yrow, xcol):
    """WE [6, n, k], A [n, k, TR], B [n, k, TC] for the given gabor subset
    over the given tile indices (float64)."""
    crk, srk = P["cr"][keep], P["sr"][keep]
    cxk, cyk = P["cx"][keep], P["cy"][keep]
    pk, qk = P["p"][keep], P["q"][keep]
    fk = P["freq"][keep]
    XcT = Xc[tiles][:, None]
    YcT = Yc[tiles][:, None]
    hxT = hx[tiles][:, None]
    hyT = hy[tiles][:, None]
    cxt = XcT * crk[None, :] + YcT * srk[None, :] + cxk[None, :]
    cyt = -XcT * srk[None, :] + YcT * crk[None, :] + cyk[None, :]
    a1 = hxT * crk[None, :]
    a2 = hyT * srk[None, :]
    b1 = -hxT * srk[None, :]
    b2 = hyT * crk[None, :]
    n, k = cxt.shape
    WE = np.empty((6, n, k))
    WE[0] = -2.0 * (pk * cxt * a1 + qk * cyt * b1)
    WE[1] = -2.0 * (pk * cxt * a2 + qk * cyt * b2)
    WE[2] = -(pk * cxt * cxt + qk * cyt * cyt)
    WE[3] = -(pk * a1 * a1 + qk * b1 * b1)
    WE[4] = -(pk * a2 * a2 + qk * b2 * b2)
    WE[5] = -2.0 * (pk * a1 * a2 + qk * b1 * b2)
    A = _wrap(fk[None, :, None] * srk[None, :, None]
              * (yrow[tiles][:, None, :] - YcT[:, :, None]))
    Bt = _wrap(fk[None, :, None] * crk[None, :, None]
               * (xcol[tiles][:, None, :] - XcT[:, :, None])
               + (fk[None, :] * cxt)[:, :, None])
    return WE, A, Bt


def _host_arrays_packed(inputs, P, gx, gy, keepLR):
    ii, jj = np.divmod(np.arange(N), TC)
    di = (ii - TR // 2).astype(np.float64)
    dj = (jj - TC // 2).astype(np.float64)
    feat6 = np.stack([dj, di, np.ones_like(dj), dj * dj, di * di, dj * di], 0)
    feat12 = np.concatenate([feat6, feat6], 0).astype(np.float32)

    onehot = np.zeros((KS, N), np.float32)
    onehot[ii, np.arange(N)] = 1.0
    onehot[TR + jj, np.arange(N)] = 1.0
    onehot[TR + TC:] = onehot[:TR + TC]

    Xc, Yc, hx, hy, yrow, xcol = _tile_geometry(gx, gy)

    # plane pl = r*8+cb -> left tile r*16+cb, right tile r*16+cb+8
    rr = np.arange(NPL) // 8
    cc = np.arange(NPL) % 8

    in_maps = []
    for core in range(NCORES):
        keepL, keepR = keepLR[core]
        base = core * NT
        tilesL = base + rr * 16 + cc
        tilesR = tilesL + 8

        we12 = np.zeros((12, NPL, 128), np.float32)
        ws = np.zeros((KS, NPL, 128))
        AB = np.zeros((128, 6))
        for side, keep, tiles in (
            (0, keepL, tilesL), (1, keepR, tilesR)
        ):
            k = len(keep)
            o = side * 64
            WE, A, Bt = _tile_tables(P, keep, tiles, Xc, Yc, hx, hy,
                                     yrow, xcol)
            WEh = _to_f32r(WE)
            WEl = _to_f32r(WE - WEh)
            we12[0:6, :, o:o + k] = WEh
            we12[6:12, :, o:o + k] = WEl
            WS = np.concatenate([A.transpose(2, 0, 1),
                                 Bt.transpose(2, 0, 1)], 0)  # [48, NPL, k]
            WSh = _to_bf16(WS).astype(np.float64)
            ws[0:48, :, o:o + k] = WSh
            ws[48:96, :, o:o + k] = WS - WSh
            AB[o:o + k, 0:3] = P["alpha"][keep]
            AB[o:o + k, 3:6] = P["beta"][keep]

        in_maps.append({
            "feat": feat12,
            "onehot": _to_bf16(onehot),
            "we": np.ascontiguousarray(we12),
            "ws": _to_bf16(ws),
            "ab": AB.astype(np.float16),
        })
    return in_maps


def kernel(**inputs):
    from concourse.bass_utils import run_bass_kernel_spmd

    gx = np.asarray(inputs["grid_x"], np.float64)
    gy = np.asarray(inputs["grid_y"], np.float64)
    P = _fold_params(inputs)

    keepLR = []
    packed = True
    for core in range(NCORES):
        rows = slice(core * SH, (core + 1) * SH)
        kL = _keeps(P, gx, gy, rows, slice(0, W // 2))
        kR = _keeps(P, gx, gy, rows, slice(W // 2, W))
        if len(kL) > 64 or len(kR) > 64:
            packed = False
        keepLR.append((kL, kR))

    if not packed:
        return _kernel_unpacked(inputs)

    in_maps = _host_arrays_packed(inputs, P, gx, gy, keepLR)
    if "packed" not in _PROGRAMS:
        _PROGRAMS["packed"] = _build_program_packed()
    nc = _PROGRAMS["packed"]
    res = run_bass_kernel_spmd(nc, in_maps, list(range(NCORES)))
    out = np.empty((3, H, W), np.float32)
    for core in range(NCORES):
        r = res.results[core]["out"]              # [2, 3, NPL, N]
        # plane pl = rowblk*8+cb; side 0 -> tile col cb, side 1 -> cb+8
        arr = r.reshape(2, 3, SH // TR, 8, TR, TC)
        out[:, core * SH:(core + 1) * SH, :] = (
            arr.transpose(1, 2, 4, 0, 3, 5).reshape(3, SH, W)
        )
    np.clip(out, -1.0, 1.0, out=out)
    return out


# ---------------------------------------------------------------------------
# Fallback: v2 per-tile program (used only if a column half keeps > 64
# gabors; correct for any input).
# ---------------------------------------------------------------------------

B_FB = 8


def _build_program_unpacked(nchunk):
    from concourse import bacc, mybir, tile

    f32 = mybir.dt.float32
    f32r = mybir.dt.float32r
    bf16 = mybir.dt.bfloat16
    f16 = mybir.dt.float16
    Act = mybir.ActivationFunctionType
    Gc = 128 * nchunk
    mmbufs = 2 if nchunk == 1 else 1
    NBLK = NT // B_FB

    nc = bacc.Bacc("TRN2", target_bir_lowering=False, debug=False,
                   num_devices=NCORES)

    featd = nc.dram_tensor("feat", [12, N], f32r, kind="ExternalInput")
    ohd = nc.dram_tensor("onehot", [KS, N], bf16, kind="ExternalInput")
    wed = nc.dram_tensor("we", [12, NT, Gc], f32r, kind="ExternalInput")
    wsd = nc.dram_tensor("ws", [KS, NT, Gc], bf16, kind="ExternalInput")
    abd = nc.dram_tensor("ab", [128, nchunk * 2 * 3], f16,
                         kind="ExternalInput")
    outd = nc.dram_tensor("out", [3, NT, N], f32, kind="ExternalOutput")

    with tile.TileContext(nc) as tc:
        with (
            tc.tile_pool(name="io", bufs=1) as iop,
            tc.tile_pool(name="gauss", bufs=B_FB // 2 + 2) as gp,
            tc.tile_pool(name="trig", bufs=3) as trigp,
            tc.tile_pool(name="prod", bufs=3) as pp,
            tc.tile_pool(name="mme", bufs=mmbufs, space="PSUM") as mmep,
            tc.tile_pool(name="mms", bufs=mmbufs, space="PSUM") as mmsp,
            tc.tile_pool(name="acc", bufs=2, space="PSUM") as accp,
        ):
            ab_sb = iop.tile([128, nchunk * 2 * 3], f16, tag="ab")
            nc.sync.dma_start(out=ab_sb[:], in_=abd[:])
            oh_sb = iop.tile([KS, N], bf16, tag="oh")
            nc.sync.dma_start(out=oh_sb[:], in_=ohd[:])
            ft_sb = iop.tile([12, N], f32r, tag="ft")
            nc.sync.dma_start(out=ft_sb[:], in_=featd[:])

            for blk in range(NBLK):
                t0 = blk * B_FB
                we = iop.tile([12, B_FB, Gc], f32r, tag="we", bufs=2)
                nc.sync.dma_start(out=we[:], in_=wed[:, t0:t0 + B_FB, :])
                ws = iop.tile([KS, B_FB, Gc], bf16, tag="ws", bufs=2)
                nc.sync.dma_start(out=ws[:], in_=wsd[:, t0:t0 + B_FB, :])

                gts = []
                for t in range(B_FB):
                    mE = mmep.tile([128, nchunk, N], f32, tag="mE", name="mE")
                    for c in range(nchunk):
                        nc.tensor.matmul(
                            mE[:, c, :],
                            we[:, t, c * 128:(c + 1) * 128],
                            ft_sb[:],
                            start=True, stop=True,
                        )
                    if t % 2 == 0:
                        gpair = gp.tile([128, 2 * nchunk, N], f16, tag="g",
                                        name="gauss")
                        gts.append(gpair)
                    nc.scalar.activation(
                        gpair[:, (t % 2) * nchunk:(t % 2 + 1) * nchunk],
                        mE[:], Act.Exp)

                for t in range(B_FB):
                    mS = mmsp.tile([128, nchunk, N], f32, tag="mS", name="mS")
                    for c in range(nchunk):
                        nc.tensor.matmul(
                            mS[:, c, :],
                            ws[:, t, c * 128:(c + 1) * 128],
                            oh_sb[:],
                            start=True, stop=True,
                        )
                    if t % 2 == 0:
                        w1p = trigp.tile([128, 2 * nchunk, N], f16, tag="w1",
                                         name="w1")
                    nc.vector.add_range_wrap(
                        w1p[:, (t % 2) * nchunk:(t % 2 + 1) * nchunk],
                        mS[:], 0.0, PI, 2.0 * PI)
                    if t % 2 == 1:
                        w2p = trigp.tile([128, 2 * nchunk, N], f16, tag="w2",
                                         name="w2")
                        nc.vector.add_range_wrap(w2p[:], w1p[:],
                                                 PI / 2, PI, 2.0 * PI)
                        ssp = trigp.tile([128, 2 * nchunk, N], f16, tag="ss",
                                         name="ss")
                        nc.scalar.activation(ssp[:], w1p[:], Act.Sin)
                        csp = trigp.tile([128, 2 * nchunk, N], f16, tag="cs",
                                         name="cs")
                        nc.scalar.activation(csp[:], w2p[:], Act.Sin)

                        gpair = gts[t // 2]
                        p1p = pp.tile([128, 2 * nchunk, N], f16, tag="p1",
                                      name="p1")
                        nc.vector.tensor_mul(p1p[:], gpair[:], csp[:])
                        p2p = pp.tile([128, 2 * nchunk, N], f16, tag="p2",
                                      name="p2")
                        nc.vector.tensor_mul(p2p[:], gpair[:], ssp[:])

                        po = accp.tile([3, 2, N], f32, tag="po", name="po")
                        for hh in range(2):
                            ops = [(p1p, c) for c in range(nchunk)] + \
                                  [(p2p, c) for c in range(nchunk)]
                            for ci, (src, c) in enumerate(ops):
                                ab_col = (0 if src is p1p
                                          else 3 * nchunk) + 3 * c
                                nc.tensor.matmul(
                                    po[:, hh],
                                    ab_sb[:, ab_col:ab_col + 3],
                                    src[:, hh * nchunk + c, :],
                                    start=(ci == 0),
                                    stop=(ci == len(ops) - 1),
                                )
                        ob = pp.tile([3, 2, N], f32, tag="ob", name="ob")
                        nc.vector.tensor_copy(ob[:], po[:])
                        nc.sync.dma_start(
                            out=outd[:, t0 + t - 1:t0 + t + 1, :],
                            in_=ob[:],
                        )

    nc.compile()
    return nc


def _kernel_unpacked(inputs):
    from concourse.bass_utils import run_bass_kernel_spmd

    gx = np.asarray(inputs["grid_x"], np.float64)
    gy = np.asarray(inputs["grid_y"], np.float64)
    P = _fold_params(inputs)

    keep_lists = []
    for core in range(NCORES):
        rows = slice(core * SH, (core + 1) * SH)
        keep_lists.append(_keeps(P, gx, gy, rows, slice(0, W)))
    gmax = max(len(k) for k in keep_lists)
    nchunk = max(1, -(-gmax // 128))
    Gc = 128 * nchunk

    ii, jj = np.divmod(np.arange(N), TC)
    di = (ii - TR // 2).astype(np.float64)
    dj = (jj - TC // 2).astype(np.float64)
    feat6 = np.stack([dj, di, np.ones_like(dj), dj * dj, di * di, dj * di], 0)
    feat12 = np.concatenate([feat6, feat6], 0).astype(np.float32)

    onehot = np.zeros((KS, N), np.float32)
    onehot[ii, np.arange(N)] = 1.0
    onehot[TR + jj, np.arange(N)] = 1.0
    onehot[TR + TC:] = onehot[:TR + TC]

    Xc, Yc, hx, hy, yrow, xcol = _tile_geometry(gx, gy)

    in_maps = []
    for core in range(NCORES):
        keep = keep_lists[core]
        k = len(keep)
        tiles = np.arange(core * NT, (core + 1) * NT)
        WE, A, Bt = _tile_tables(P, keep, tiles, Xc, Yc, hx, hy, yrow, xcol)

        we12 = np.zeros((12, NT, Gc), np.float32)
        WEh = _to_f32r(WE)
        we12[0:6, :, :k] = WEh
        we12[6:12, :, :k] = _to_f32r(WE - WEh)

        ws = np.zeros((KS, NT, Gc))
        WS = np.concatenate([A.transpose(2, 0, 1), Bt.transpose(2, 0, 1)], 0)
        WSh = _to_bf16(WS).astype(np.float64)
        ws[0:48, :, :k] = WSh
        ws[48:96, :, :k] = WS - WSh

        AB = np.zeros((128, nchunk * 2 * 3))
        al = np.zeros((Gc, 3)); bt = np.zeros((Gc, 3))
        al[:k] = P["alpha"][keep]
        bt[:k] = P["beta"][keep]
        for c in range(nchunk):
            AB[:, 3 * c:3 * c + 3] = al[c * 128:(c + 1) * 128]
            off = 3 * (nchunk + c)
            AB[:, off:off + 3] = bt[c * 128:(c + 1) * 128]

        in_maps.append({
            "feat": feat12,
            "onehot": _to_bf16(onehot),
            "we": np.ascontiguousarray(we12),
            "ws": _to_bf16(ws),
            "ab": AB.astype(np.float16),
        })

    key = ("unpacked", nchunk)
    if key not in _PROGRAMS:
        _PROGRAMS[key] = _build_program_unpacked(nchunk)
    nc = _PROGRAMS[key]
    res = run_bass_kernel_spmd(nc, in_maps, list(range(NCORES)))
    out = np.empty((3, H, W), np.float32)
    for core in range(NCORES):
        r = res.results[core]["out"]              # [3, NT, N]
        out[:, core * SH:(core + 1) * SH, :] = (
            r.reshape(3, SH // TR, TPR, TR, TC)
             .transpose(0, 1, 3, 2, 4)
             .reshape(3, SH, W)
        )
    np.clip(out, -1.0, 1.0, out=out)
    return out


# revision 12
# speedup vs baseline: 2.6342x; 1.1154x over previous
"""Gabor layer Trainium2 kernel (v3: packed planes).

Per gabor g and pixel (x,y): amp[g,c] * exp(E) * cos(S + phase[g,c]).
cos(S+p) = cos(p)cos(S) - sin(p)sin(S) turns the channel sum over g into
matmuls over gauss*cos(S) / gauss*sin(S) planes (contraction = gabors).

All elementwise engine costs scale with the free (pixel) axis only, so the
partition axis is free parallelism. Each 64-row strip is culled per column
half; with kL,kR <= 64 two tiles (one left-half, one right-half) pack into
one 128-partition plane, halving every exp/sin/wrap/multiply:
  plane pl = (row_block, col_block<8): partitions 0:64 carry the left
  tile's gabors, 64:128 the right tile's (tile tR = tL + 8).

Per plane:  E = [WEh;WEl]^T @ feat12     (1 f32r matmul; integer tile-local
            features [dj,di,1,dj2,di2,dj*di] are exact in f32r, weights
            hi/lo split; f32r runs 1 cycle/row vs 4 for fp32 and the PE
            product is exact for pre-rounded inputs)
            S = [Ah;Bh;Al;Bl]^T @ onehot96   (1 bf16 matmul, K=96)
            gauss = Exp(E) fp16; w1 = wrap(S); w2 = wrap(S+pi/2) (DVE);
            ss,cs = Sin(w1),Sin(w2) fp16; p1 = cs*gauss, p2 = ss*gauss
            (fp16 DVE 2x mode)
Output: 4 logical tiles accumulate into ONE PSUM bank at partition offsets
0/32/64/96 (tile_position column tiling, one accumulation group per bank),
then one 512-cycle DVE copy + 2 DMAs per quad.

Two global phases (all Exps, then all Sins) keep the Exp/Sin activation
tables from thrashing: they live in different hardware table sets and each
swap costs 1.3us (the v2 interleaved phasing measured 36 loads = 46us).

Sharding: 8 cores x 64-row strips, no collectives; clamp + reassembly on
host. Falls back to the v2 per-tile program if a column half keeps > 64
gabors (not the case for the reference inputs: kL<=55, kR<=64).
"""

import os
import sys

import numpy as np

for _p in ("/opt/trn_rl_repo",):
    if os.path.isdir(_p) and _p not in sys.path:
        sys.path.append(_p)

H = W = 512
G = 256
NCORES = 8
SH = H // NCORES      # strip rows per core
TR, TC = 16, 32       # tile rows x cols
N = TR * TC           # 512 pixels per tile
TPR = W // TC         # tiles per strip row = 16
NT = (SH // TR) * TPR # tiles per core = 64
NPL = NT // 2         # packed planes per core = 32
KS = 2 * (TR + TC)    # one-hot rows: [rowhi, colhi, rowlo, collo] = 96
PI = float(np.pi)
CULL_THR = 1e-7

_PROGRAMS = {}


def _build_program_packed():
    from concourse import bacc, mybir, tile

    f32 = mybir.dt.float32
    f32r = mybir.dt.float32r
    bf16 = mybir.dt.bfloat16
    f16 = mybir.dt.float16
    Act = mybir.ActivationFunctionType

    nc = bacc.Bacc("TRN2", target_bir_lowering=False, debug=False,
                   num_devices=NCORES)

    featd = nc.dram_tensor("feat", [12, N], f32r, kind="ExternalInput")
    ohd = nc.dram_tensor("onehot", [KS, N], bf16, kind="ExternalInput")
    wed = nc.dram_tensor("we", [12, NPL, 128], f32r, kind="ExternalInput")
    wsd = nc.dram_tensor("ws", [KS, NPL, 128], bf16, kind="ExternalInput")
    abd = nc.dram_tensor("ab", [128, 6], f16, kind="ExternalInput")
    outd = nc.dram_tensor("out", [2, 3, NPL, N], f32, kind="ExternalOutput")

    with tile.TileContext(nc) as tc:
        with (
            tc.tile_pool(name="io", bufs=1) as iop,
            tc.tile_pool(name="gauss", bufs=NPL // 2 + 1) as gp,
            tc.tile_pool(name="trig", bufs=3) as trigp,
            tc.tile_pool(name="prod", bufs=3) as pp,
            tc.tile_pool(name="mm", bufs=2, space="PSUM") as mmp,
            tc.tile_pool(name="acc", bufs=2, space="PSUM") as accp,
        ):
            ab_sb = iop.tile([128, 6], f16, tag="ab")
            nc.sync.dma_start(out=ab_sb[:], in_=abd[:])
            oh_sb = iop.tile([KS, N], bf16, tag="oh")
            nc.sync.dma_start(out=oh_sb[:], in_=ohd[:])
            ft_sb = iop.tile([12, N], f32r, tag="ft")
            nc.sync.dma_start(out=ft_sb[:], in_=featd[:])
            we = iop.tile([12, NPL, 128], f32r, tag="we")
            nc.sync.dma_start(out=we[:], in_=wed[:])
            ws = iop.tile([KS, NPL, 128], bf16, tag="ws")
            nc.sync.dma_start(out=ws[:], in_=wsd[:])

            # Phase A: all gaussians (Exp table stays loaded).
            gts = []
            for q in range(NPL // 2):
                mEp = mmp.tile([128, 2, N], f32, tag="mm", name="mE")
                for h in range(2):
                    nc.tensor.matmul(
                        mEp[:, h, :],
                        we[:, 2 * q + h, :],
                        ft_sb[:],
                        start=True, stop=True,
                    )
                gpair = gp.tile([128, 2, N], f16, tag="g", name="gauss")
                gts.append(gpair)
                nc.scalar.activation(gpair[:], mEp[:], Act.Exp)

            # Phase B: sinusoids + products + reduction (Sin table).
            # w1/w2/sin/cos batch 2 plane-pairs (4 planes) per ACT op.
            w1q = w2q = ssq = csq = None
            for q in range(NPL // 2):
                mSp = mmp.tile([128, 2, N], f32, tag="mm", name="mS")
                for h in range(2):
                    nc.tensor.matmul(
                        mSp[:, h, :],
                        ws[:, 2 * q + h, :],
                        oh_sb[:],
                        start=True, stop=True,
                    )
                if q % 2 == 0:
                    w1q = trigp.tile([128, 4, N], f16, tag="w1", name="w1")
                    w2q = trigp.tile([128, 4, N], f16, tag="w2", name="w2")
                hq = 2 * (q % 2)
                nc.vector.add_range_wrap(w1q[:, hq:hq + 2], mSp[:],
                                         0.0, PI, 2.0 * PI)
                nc.vector.add_range_wrap(w2q[:, hq:hq + 2], w1q[:, hq:hq + 2],
                                         PI / 2, PI, 2.0 * PI)
                if q % 2 == 1:
                    ssq = trigp.tile([128, 4, N], f16, tag="ss", name="ss")
                    nc.scalar.activation(ssq[:], w1q[:], Act.Sin)
                    csq = trigp.tile([128, 4, N], f16, tag="cs", name="cs")
                    nc.scalar.activation(csq[:], w2q[:], Act.Sin)
                    for qq in (q - 1, q):
                        hh = 2 * (qq % 2)
                        gpair = gts[qq]
                        p1p = pp.tile([128, 2, N], f16, tag="p1", name="p1")
                        nc.vector.tensor_mul(p1p[:], gpair[:],
                                             csq[:, hh:hh + 2])
                        p2p = pp.tile([128, 2, N], f16, tag="p2", name="p2")
                        nc.vector.tensor_mul(p2p[:], gpair[:],
                                             ssq[:, hh:hh + 2])
                        _emit_reduce(nc, accp, pp, ab_sb, outd, p1p, p2p, qq)

    nc.compile()
    return nc


def _emit_reduce(nc, accp, pp, ab_sb, outd, p1p, p2p, q):
    """2 logical tiles per PSUM bank at partition offsets 0/32 (base 96
    is illegal - PE quadrant 3), bank h = plane h of the pair; one
    accumulation group per (bank, region)."""
    from concourse import mybir
    f32 = mybir.dt.float32
    N_ = p1p.shape[-1]
    po = accp.tile([128, 2, N_], f32, tag="po", name="po")
    for h in range(2):        # plane within pair = bank
        for s in range(2):    # side: 0=left(K 0:64) 1=right
            ks, co = s * 64, s * 32
            for pi_, (src, acol) in enumerate(((p1p, 0), (p2p, 3))):
                nc.tensor.matmul(
                    po[co:co + 3, h, :],
                    ab_sb[ks:ks + 64, acol:acol + 3],
                    src[ks:ks + 64, h, :],
                    start=(pi_ == 0), stop=(pi_ == 1),
                    skip_group_check=True,
                )
    ob = pp.tile([128, 2, N_], f32, tag="ob", name="ob")
    nc.scalar.copy(ob[:], po[:])
    nc.sync.dma_start(out=outd[0, :, 2 * q:2 * q + 2, :], in_=ob[0:3, :, :])
    nc.sync.dma_start(out=outd[1, :, 2 * q:2 * q + 2, :], in_=ob[32:35, :, :])


def _wrap(x):
    return np.mod(x + np.pi, 2.0 * np.pi) - np.pi


def _to_f32r(a):
    b = np.ascontiguousarray(a, np.float32).view(np.uint32)
    r = (b + np.uint32(0x7FF) + ((b >> np.uint32(12)) & np.uint32(1))) \
        & np.uint32(0xFFFFF000)
    return r.view(np.float32)


def _to_bf16(a):
    import ml_dtypes
    return np.ascontiguousarray(a.astype(ml_dtypes.bfloat16))


def _fold_params(inputs):
    u = np.clip(np.asarray(inputs["u"], np.float64), -1, 1)
    v = np.clip(np.asarray(inputs["v"], np.float64), -1, 1)
    th = np.clip(np.asarray(inputs["theta"], np.float64), -2, 2) * (2 * np.pi)
    sig = np.clip(np.asarray(inputs["rel_sigma"], np.float64), 0.001, 1.0)
    rf = np.clip(np.asarray(inputs["rel_freq"], np.float64), -5, 5)
    gam = np.clip(np.asarray(inputs["gamma"], np.float64), 0.0001, 1.0)
    psi = np.clip(np.asarray(inputs["psi"], np.float64), -1, 1)
    amp = np.clip(np.asarray(inputs["amplitude"], np.float64), 0, 1)
    cr, sr = np.cos(th), np.sin(th)
    return dict(
        u=u, v=v, cr=cr, sr=sr,
        cx=-(cr * u + sr * v), cy=sr * u - cr * v,
        p=1.0 / (2.0 * sig * sig), q=1.0 / (2.0 * gam * gam),
        freq=2 * np.pi / np.exp(rf),
        alpha=amp * np.cos(psi * 2 * np.pi),
        beta=-amp * np.sin(psi * 2 * np.pi),
        amp=amp,
    )


def _keeps(P, gx, gy, rows, cols):
    """Exact per-pixel cull: keep gabors whose max E over the region
    clears the contribution threshold."""
    ampmax = P["amp"].max(1)
    elim = np.log(np.maximum(CULL_THR / np.maximum(ampmax, 1e-30),
                             1e-300)) - 1.0
    crf = P["cr"].astype(np.float32)[:, None]
    srf = P["sr"].astype(np.float32)[:, None]
    pf = P["p"].astype(np.float32)[:, None]
    qf = P["q"].astype(np.float32)[:, None]
    Xs = np.asarray(gx[rows][:, cols], np.float32).ravel()[None, :]
    Ys = np.asarray(gy[rows][:, cols], np.float32).ravel()[None, :]
    dx = Xs - P["u"].astype(np.float32)[:, None]
    dy = Ys - P["v"].astype(np.float32)[:, None]
    xr = dx * crf + dy * srf
    yr = dy * crf - dx * srf
    quad = xr * xr * pf
    quad += yr * yr * qf
    Em = -quad.min(1)
    return np.flatnonzero(Em >= elim)


def _tile_geometry(gx, gy):
    """Tile-major grids and per-tile affine centers/steps."""
    Xt = gx.reshape(H // TR, TR, W // TC, TC).transpose(0, 2, 1, 3).reshape(-1, N)
    Yt = gy.reshape(H // TR, TR, W // TC, TC).transpose(0, 2, 1, 3).reshape(-1, N)
    hx = Xt[:, 1] - Xt[:, 0]
    hy = Yt[:, TC] - Yt[:, 0]
    Xc = Xt[:, TR // 2 * TC + TC // 2]
    Yc = Yt[:, TR // 2 * TC + TC // 2]
    yrow = Yt.reshape(-1, TR, TC)[:, :, 0]
    xcol = Xt.reshape(-1, TR, TC)[:, 0, :]
    return Xc, Yc, hx, hy, yrow, xcol


def _tile_tables(P, keep, tiles, Xc, Yc, hx, hy, yrow, xcol):
    """WE [6, n, k], A [n, k, TR], B [n, k, TC] for the given gabor subset
    over the given tile indices (float64)."""
    crk, srk = P["cr"][keep], P["sr"][keep]
    cxk, cyk = P["cx"][keep], P["cy"][keep]
    pk, qk = P["p"][keep], P["q"][keep]
    fk = P["freq"][keep]
    XcT = Xc[tiles][:, None]
    YcT = Yc[tiles][:, None]
    hxT = hx[tiles][:, None]
    hyT = hy[tiles][:, None]
    cxt = XcT * crk[None, :] + YcT * srk[None, :] + cxk[None, :]
    cyt = -XcT * srk[None, :] + YcT * crk[None, :] + cyk[None, :]
    a1 = hxT * crk[None, :]
    a2 = hyT * srk[None, :]
    b1 = -hxT * srk[None, :]
    b2 = hyT * crk[None, :]
    n, k = cxt.shape
    WE = np.empty((6, n, k))
    WE[0] = -2.0 * (pk * cxt * a1 + qk * cyt * b1)
    WE[1] = -2.0 * (pk * cxt * a2 + qk * cyt * b2)
    WE[2] = -(pk * cxt * cxt + qk * cyt * cyt)
    WE[3] = -(pk * a1 * a1 + qk * b1 * b1)
    WE[4] = -(pk * a2 * a2 + qk * b2 * b2)
    WE[5] = -2.0 * (pk * a1 * a2 + qk * b1 * b2)
    A = _wrap(fk[None, :, None] * srk[None, :, None]
              * (yrow[tiles][:, None, :] - YcT[:, :, None]))
    Bt = _wrap(fk[None, :, None] * crk[None, :, None]
               * (xcol[tiles][:, None, :] - XcT[:, :, None])
               + (fk[None, :] * cxt)[:, :, None])
    return WE, A, Bt


def _host_arrays_packed(inputs, P, gx, gy, keepLR):
    ii, jj = np.divmod(np.arange(N), TC)
    di = (ii - TR // 2).astype(np.float64)
    dj = (jj - TC // 2).astype(np.float64)
    feat6 = np.stack([dj, di, np.ones_like(dj), dj * dj, di * di, dj * di], 0)
    feat12 = np.concatenate([feat6, feat6], 0).astype(np.float32)

    onehot = np.zeros((KS, N), np.float32)
    onehot[ii, np.arange(N)] = 1.0
    onehot[TR + jj, np.arange(N)] = 1.0
    onehot[TR + TC:] = onehot[:TR + TC]

    Xc, Yc, hx, hy, yrow, xcol = _tile_geometry(gx, gy)

    # plane pl = r*8+cb -> left tile r*16+cb, right tile r*16+cb+8
    rr = np.arange(NPL) // 8
    cc = np.arange(NPL) % 8

    in_maps = []
    for core in range(NCORES):
        keepL, keepR = keepLR[core]
        base = core * NT
        tilesL = base + rr * 16 + cc
        tilesR = tilesL + 8

        we12 = np.zeros((12, NPL, 128), np.float32)
        ws = np.zeros((KS, NPL, 128))
        AB = np.zeros((128, 6))
        for side, keep, tiles in (
            (0, keepL, tilesL), (1, keepR, tilesR)
        ):
            k = len(keep)
            o = side * 64
            WE, A, Bt = _tile_tables(P, keep, tiles, Xc, Yc, hx, hy,
                                     yrow, xcol)
            WEh = _to_f32r(WE)
            WEl = _to_f32r(WE - WEh)
            we12[0:6, :, o:o + k] = WEh
            we12[6:12, :, o:o + k] = WEl
            WS = np.concatenate([A.transpose(2, 0, 1),
                                 Bt.transpose(2, 0, 1)], 0)  # [48, NPL, k]
            WSh = _to_bf16(WS).astype(np.float64)
            ws[0:48, :, o:o + k] = WSh
            ws[48:96, :, o:o + k] = WS - WSh
            AB[o:o + k, 0:3] = P["alpha"][keep]
            AB[o:o + k, 3:6] = P["beta"][keep]

        in_maps.append({
            "feat": feat12,
            "onehot": _to_bf16(onehot),
            "we": np.ascontiguousarray(we12),
            "ws": _to_bf16(ws),
            "ab": AB.astype(np.float16),
        })
    return in_maps


def kernel(**inputs):
    from concourse.bass_utils import run_bass_kernel_spmd

    gx = np.asarray(inputs["grid_x"], np.float64)
    gy = np.asarray(inputs["grid_y"], np.float64)
    P = _fold_params(inputs)

    keepLR = []
    packed = True
    for core in range(NCORES):
        rows = slice(core * SH, (core + 1) * SH)
        kL = _keeps(P, gx, gy, rows, slice(0, W // 2))
        kR = _keeps(P, gx, gy, rows, slice(W // 2, W))
        if len(kL) > 64 or len(kR) > 64:
            packed = False
        keepLR.append((kL, kR))

    if not packed:
        return _kernel_unpacked(inputs)

    in_maps = _host_arrays_packed(inputs, P, gx, gy, keepLR)
    if "packed" not in _PROGRAMS:
        _PROGRAMS["packed"] = _build_program_packed()
    nc = _PROGRAMS["packed"]
    res = run_bass_kernel_spmd(nc, in_maps, list(range(NCORES)))
    out = np.empty((3, H, W), np.float32)
    for core in range(NCORES):
        r = res.results[core]["out"]              # [2, 3, NPL, N]
        # plane pl = rowblk*8+cb; side 0 -> tile col cb, side 1 -> cb+8
        arr = r.reshape(2, 3, SH // TR, 8, TR, TC)
        out[:, core * SH:(core + 1) * SH, :] = (
            arr.transpose(1, 2, 4, 0, 3, 5).reshape(3, SH, W)
        )
    np.clip(out, -1.0, 1.0, out=out)
    return out


# ---------------------------------------------------------------------------
# Fallback: v2 per-tile program (used only if a column half keeps > 64
# gabors; correct for any input).
# ---------------------------------------------------------------------------

B_FB = 8


def _build_program_unpacked(nchunk):
    from concourse import bacc, mybir, tile

    f32 = mybir.dt.float32
    f32r = mybir.dt.float32r
    bf16 = mybir.dt.bfloat16
    f16 = mybir.dt.float16
    Act = mybir.ActivationFunctionType
    Gc = 128 * nchunk
    mmbufs = 2 if nchunk == 1 else 1
    NBLK = NT // B_FB

    nc = bacc.Bacc("TRN2", target_bir_lowering=False, debug=False,
                   num_devices=NCORES)

    featd = nc.dram_tensor("feat", [12, N], f32r, kind="ExternalInput")
    ohd = nc.dram_tensor("onehot", [KS, N], bf16, kind="ExternalInput")
    wed = nc.dram_tensor("we", [12, NT, Gc], f32r, kind="ExternalInput")
    wsd = nc.dram_tensor("ws", [KS, NT, Gc], bf16, kind="ExternalInput")
    abd = nc.dram_tensor("ab", [128, nchunk * 2 * 3], f16,
                         kind="ExternalInput")
    outd = nc.dram_tensor("out", [3, NT, N], f32, kind="ExternalOutput")

    with tile.TileContext(nc) as tc:
        with (
            tc.tile_pool(name="io", bufs=1) as iop,
            tc.tile_pool(name="gauss", bufs=B_FB // 2 + 2) as gp,
            tc.tile_pool(name="trig", bufs=3) as trigp,
            tc.tile_pool(name="prod", bufs=3) as pp,
            tc.tile_pool(name="mme", bufs=mmbufs, space="PSUM") as mmep,
            tc.tile_pool(name="mms", bufs=mmbufs, space="PSUM") as mmsp,
            tc.tile_pool(name="acc", bufs=2, space="PSUM") as accp,
        ):
            ab_sb = iop.tile([128, nchunk * 2 * 3], f16, tag="ab")
            nc.sync.dma_start(out=ab_sb[:], in_=abd[:])
            oh_sb = iop.tile([KS, N], bf16, tag="oh")
            nc.sync.dma_start(out=oh_sb[:], in_=ohd[:])
            ft_sb = iop.tile([12, N], f32r, tag="ft")
            nc.sync.dma_start(out=ft_sb[:], in_=featd[:])

            for blk in range(NBLK):
                t0 = blk * B_FB
                we = iop.tile([12, B_FB, Gc], f32r, tag="we", bufs=2)
                nc.sync.dma_start(out=we[:], in_=wed[:, t0:t0 + B_FB, :])
                ws = iop.tile([KS, B_FB, Gc], bf16, tag="ws", bufs=2)
                nc.sync.dma_start(out=ws[:], in_=wsd[:, t0:t0 + B_FB, :])

                gts = []
                for t in range(B_FB):
                    mE = mmep.tile([128, nchunk, N], f32, tag="mE", name="mE")
                    for c in range(nchunk):
                        nc.tensor.matmul(
                            mE[:, c, :],
                            we[:, t, c * 128:(c + 1) * 128],
                            ft_sb[:],
                            start=True, stop=True,
                        )
                    if t % 2 == 0:
                        gpair = gp.tile([128, 2 * nchunk, N], f16, tag="g",
                                        name="gauss")
                        gts.append(gpair)
                    nc.scalar.activation(
                        gpair[:, (t % 2) * nchunk:(t % 2 + 1) * nchunk],
                        mE[:], Act.Exp)

                for t in range(B_FB):
                    mS = mmsp.tile([128, nchunk, N], f32, tag="mS", name="mS")
                    for c in range(nchunk):
                        nc.tensor.matmul(
                            mS[:, c, :],
                            ws[:, t, c * 128:(c + 1) * 128],
                            oh_sb[:],
                            start=True, stop=True,
                        )
                    if t % 2 == 0:
                        w1p = trigp.tile([128, 2 * nchunk, N], f16, tag="w1",
                                         name="w1")
                    nc.vector.add_range_wrap(
                        w1p[:, (t % 2) * nchunk:(t % 2 + 1) * nchunk],
                        mS[:], 0.0, PI, 2.0 * PI)
                    if t % 2 == 1:
                        w2p = trigp.tile([128, 2 * nchunk, N], f16, tag="w2",
                                         name="w2")
                        nc.vector.add_range_wrap(w2p[:], w1p[:],
                                                 PI / 2, PI, 2.0 * PI)
                        ssp = trigp.tile([128, 2 * nchunk, N], f16, tag="ss",
                                         name="ss")
                        nc.scalar.activation(ssp[:], w1p[:], Act.Sin)
                        csp = trigp.tile([128, 2 * nchunk, N], f16, tag="cs",
                                         name="cs")
                        nc.scalar.activation(csp[:], w2p[:], Act.Sin)

                        gpair = gts[t // 2]
                        p1p = pp.tile([128, 2 * nchunk, N], f16, tag="p1",
                                      name="p1")
                        nc.vector.tensor_mul(p1p[:], gpair[:], csp[:])
                        p2p = pp.tile([128, 2 * nchunk, N], f16, tag="p2",
                                      name="p2")
                        nc.vector.tensor_mul(p2p[:], gpair[:], ssp[:])

                        po = accp.tile([3, 2, N], f32, tag="po", name="po")
                        for hh in range(2):
                            ops = [(p1p, c) for c in range(nchunk)] + \
                                  [(p2p, c) for c in range(nchunk)]
                            for ci, (src, c) in enumerate(ops):
                                ab_col = (0 if src is p1p
                                          else 3 * nchunk) + 3 * c
                                nc.tensor.matmul(
                                    po[:, hh],
                                    ab_sb[:, ab_col:ab_col + 3],
                                    src[:, hh * nchunk + c, :],
                                    start=(ci == 0),
                                    stop=(ci == len(ops) - 1),
                                )
                        ob = pp.tile([3, 2, N], f32, tag="ob", name="ob")
                        nc.vector.tensor_copy(ob[:], po[:])
                        nc.sync.dma_start(
                            out=outd[:, t0 + t - 1:t0 + t + 1, :],
                            in_=ob[:],
                        )

    nc.compile()
    return nc


def _kernel_unpacked(inputs):
    from concourse.bass_utils import run_bass_kernel_spmd

    gx = np.asarray(inputs["grid_x"], np.float64)
    gy = np.asarray(inputs["grid_y"], np.float64)
    P = _fold_params(inputs)

    keep_lists = []
    for core in range(NCORES):
        rows = slice(core * SH, (core + 1) * SH)
        keep_lists.append(_keeps(P, gx, gy, rows, slice(0, W)))
    gmax = max(len(k) for k in keep_lists)
    nchunk = max(1, -(-gmax // 128))
    Gc = 128 * nchunk

    ii, jj = np.divmod(np.arange(N), TC)
    di = (ii - TR // 2).astype(np.float64)
    dj = (jj - TC // 2).astype(np.float64)
    feat6 = np.stack([dj, di, np.ones_like(dj), dj * dj, di * di, dj * di], 0)
    feat12 = np.concatenate([feat6, feat6], 0).astype(np.float32)

    onehot = np.zeros((KS, N), np.float32)
    onehot[ii, np.arange(N)] = 1.0
    onehot[TR + jj, np.arange(N)] = 1.0
    onehot[TR + TC:] = onehot[:TR + TC]

    Xc, Yc, hx, hy, yrow, xcol = _tile_geometry(gx, gy)

    in_maps = []
    for core in range(NCORES):
        keep = keep_lists[core]
        k = len(keep)
        tiles = np.arange(core * NT, (core + 1) * NT)
        WE, A, Bt = _tile_tables(P, keep, tiles, Xc, Yc, hx, hy, yrow, xcol)

        we12 = np.zeros((12, NT, Gc), np.float32)
        WEh = _to_f32r(WE)
        we12[0:6, :, :k] = WEh
        we12[6:12, :, :k] = _to_f32r(WE - WEh)

        ws = np.zeros((KS, NT, Gc))
        WS = np.concatenate([A.transpose(2, 0, 1), Bt.transpose(2, 0, 1)], 0)
        WSh = _to_bf16(WS).astype(np.float64)
        ws[0:48, :, :k] = WSh
        ws[48:96, :, :k] = WS - WSh

        AB = np.zeros((128, nchunk * 2 * 3))
        al = np.zeros((Gc, 3)); bt = np.zeros((Gc, 3))
        al[:k] = P["alpha"][keep]
        bt[:k] = P["beta"][keep]
        for c in range(nchunk):
            AB[:, 3 * c:3 * c + 3] = al[c * 128:(c + 1) * 128]
            off = 3 * (nchunk + c)
            AB[:, off:off + 3] = bt[c * 128:(c + 1) * 128]

        in_maps.append({
            "feat": feat12,
            "onehot": _to_bf16(onehot),
            "we": np.ascontiguousarray(we12),
            "ws": _to_bf16(ws),
            "ab": AB.astype(np.float16),
        })

    key = ("unpacked", nchunk)
    if key not in _PROGRAMS:
        _PROGRAMS[key] = _build_program_unpacked(nchunk)
    nc = _PROGRAMS[key]
    res = run_bass_kernel_spmd(nc, in_maps, list(range(NCORES)))
    out = np.empty((3, H, W), np.float32)
    for core in range(NCORES):
        r = res.results[core]["out"]              # [3, NT, N]
        out[:, core * SH:(core + 1) * SH, :] = (
            r.reshape(3, SH // TR, TPR, TR, TC)
             .transpose(0, 1, 3, 2, 4)
             .reshape(3, SH, W)
        )
    np.clip(out, -1.0, 1.0, out=out)
    return out
